# revision 1
# baseline (speedup 1.0000x reference)
"""Bilinear pooling kernel for 8 Trainium2 NeuronCores (Bass/Tile).

Math (matches the jax reference):
  x = concat([x1, x2, x3], channel) -> (B=64, M=147, L=3136)
  phi_b = x_b @ x_b.T                              (147, 147), symmetric
  phi = sign(phi) * sqrt(|phi| + EPS)              (signed sqrt)
  phi = phi / sqrt(sum(phi^2 + EPS) + 1.0)         (per-batch normalize)
  h = phi_vec @ fc0_w.T + fc0_b                    (64, 1024)
  y = h @ fc1_w.T + fc1_b                          (64, 64)
  logits = y @ fc2_w.T + fc2_b                     (64, 4)
  merged = softmax(concat([logits, x11, x21, x31]))
  x_merge = merged @ cls_w.T + cls_b               (64, 4)
  returns (logits, x_merge)

Distribution:
  phase 0: preload all fc0 weights + tail constants (overlaps phase 1)
  phase 1: batch-parallel bilinear+signed-sqrt+normalize (8 batches/core)
  phase 2: AllGather normalized phi (padded to 21632 cols)
  phase 3: PE-transpose phi to (i, b) layout; fc0 with output-column-sharded
           W^T (128 of 1024 outputs per core); fc1 partial contraction
  phase 4: AllReduce the (64, 64) y^T partials
  phase 5: replicated tail (fc2, softmax, cls); outputs read from core 0

MM_DT selects the matmul dtype for the two big GEMMs (bilinear + fc0);
everything else (signed sqrt, normalize, fc1/fc2/softmax/cls) stays fp32.
"""

import sys

sys.path.insert(0, "/opt/trn_rl_repo")

import numpy as np

import concourse.bass as bass
import concourse.tile as tile
from concourse import masks, mybir
from concourse.bass_utils import run_bass_kernel_spmd
import bass_rust
from bass_rust import ScopedClock

# ---------------------------------------------------------------------------
# Workaround: this toolchain's walrus accepts only ONE semaphore wait per
# instruction, but Tile can attach several.  Split excess waits onto
# same-engine nops placed immediately before the instruction (same engine
# => executed in order, so synchronization semantics are unchanged).
# ---------------------------------------------------------------------------
_MAX_WAITS = 1
_ws_counter = [0]


def _split_excess_waits(obb):
    for bb, insts in list(obb.items()):
        new_list = []
        for inst in insts:
            info = inst.sync_info
            if info is not None and len(info.on_wait) > _MAX_WAITS:
                waits = list(info.on_wait)
                excess = waits[:-_MAX_WAITS]
                keep = waits[-_MAX_WAITS:]
                for i in range(0, len(excess), _MAX_WAITS):
                    _ws_counter[0] += 1
                    nop = mybir.InstNoOp(
                        name=f"WS-{_ws_counter[0]}",
                        sync_info=bass_rust.SyncInfo(
                            on_wait=excess[i : i + _MAX_WAITS], on_update=[]
                        ),
                        bass_nofuse=True,
                        engine=inst.engine,
                    )
                    new_list.append(nop)
                inst.sync_info = bass_rust.SyncInfo(
                    on_wait=keep, on_update=list(info.on_update)
                )
            new_list.append(inst)
        obb[bb] = new_list


_RealTCW = tile.TileClockWait


class _TCWWrapper:
    def __init__(self, *args, **kwargs):
        self._inner = _RealTCW(*args, **kwargs)
        self._obb = (
            args[1] if len(args) > 1 else kwargs["ordered_instructions_by_block"]
        )

    def __getattr__(self, name):
        return getattr(self._inner, name)

    def assign_waits(self, bb_name):
        self._inner.assign_waits(bb_name)
        _split_excess_waits(self._obb)


tile.TileClockWait = _TCWWrapper


def _split_drain_and_barrier(self, tick_clock, wait_clock):
    nc = self.nc
    drain_inst = nc.sync.drain()
    wait_clock.add_sem_waits(
        drain_inst.ins, ScopedClock({None: tick_clock.global_clock})
    )
    info = drain_inst.ins.sync_info
    if info is not None and len(info.on_wait) > _MAX_WAITS:
        waits = list(info.on_wait)
        drain_inst.ins.sync_info = bass_rust.SyncInfo(
            on_wait=waits[:_MAX_WAITS], on_update=list(info.on_update)
        )
        rest = waits[_MAX_WAITS:]
        while rest:
            chunk, rest = rest[:_MAX_WAITS], rest[_MAX_WAITS:]
            nop_inst = nc.sync.nop(nofuse=True, hint="tail_drain_split")
            nop_inst.ins.sync_info = bass_rust.SyncInfo(on_wait=chunk, on_update=[])
    nc.all_engine_barrier()
    assert self.sems is not None
    popped = nc._tile_sem_poison_stack.pop()
    assert popped is self._sem_poison
    nc.clear_and_free_semaphores(list(self.sems.allocated().values()))
    nc.all_engine_barrier()


tile.TileContext._drain_and_barrier = _split_drain_and_barrier

# ---------------------------------------------------------------------------
# Problem constants (hardcoded per the spec)
# ---------------------------------------------------------------------------
N_CORES = 8
CORE_IDS = list(range(N_CORES))
B = 64
B_LOC = B // N_CORES  # 8 batches per core
C = 49
L = 3136  # 56*56
M = 147  # 3*49 channels
MM = M * M  # 21609
NI_CHUNKS = 169  # ceil(MM/128)
MM_PAD = NI_CHUNKS * 128  # 21632
O0 = 1024  # fc0 out features
O0_LOC = O0 // N_CORES  # 128 per core
HID = 64  # fc1 out features
CLS = 4
EPS = 1e-8
# normalizer constant: sum(phi_ss^2 + EPS) + 1.0 == sum|phi| + 2*MM*EPS + 1.0
NORM_C = float(2 * MM * EPS + 1.0)

LFULL = 24  # full 128-row l-chunks
LTAIL = 64  # tail chunk rows (3136 = 24*128 + 64)

F32 = mybir.dt.float32

# matmul dtype for the two big GEMMs: "float32", "bfloat16", or "float16"
MM_DT_NAME = "float16"
MM_DT = getattr(mybir.dt, MM_DT_NAME)
W_DMA = 8  # i-chunks per fc0 weight DMA


def _build_nc():
    nc = bass.Bass()

    # -- external I/O ------------------------------------------------------
    # x arrives host-side concatenated over channels and transposed to
    # (b, l, m) so device loads are contiguous along the innermost dim.
    xall_d = nc.dram_tensor("xall", [B_LOC, L, M], MM_DT, kind="ExternalInput")
    x11_d = nc.dram_tensor("x11", [B, CLS], F32, kind="ExternalInput")
    x21_d = nc.dram_tensor("x21", [B, CLS], F32, kind="ExternalInput")
    x31_d = nc.dram_tensor("x31", [B, CLS], F32, kind="ExternalInput")
    w0t_d = nc.dram_tensor("w0t", [MM_PAD, O0_LOC], MM_DT, kind="ExternalInput")
    fc0b_d = nc.dram_tensor("fc0b", [O0_LOC, 1], F32, kind="ExternalInput")
    w1t_d = nc.dram_tensor("w1t", [O0_LOC, HID], F32, kind="ExternalInput")
    fc1b_d = nc.dram_tensor("fc1b", [HID, 1], F32, kind="ExternalInput")
    w2t_d = nc.dram_tensor("w2t", [HID + 1, CLS], F32, kind="ExternalInput")
    wct_d = nc.dram_tensor("wct", [4 * CLS + 1, CLS], F32, kind="ExternalInput")
    logits_d = nc.dram_tensor("logits", [B, CLS], F32, kind="ExternalOutput")
    xmerge_d = nc.dram_tensor("x_merge", [B, CLS], F32, kind="ExternalOutput")

    n_wdma = (NI_CHUNKS + W_DMA - 1) // W_DMA  # 22 (last has 1 chunk)

    with tile.TileContext(nc) as tc:
        with tc.tile_pool(name="dram", bufs=1, space="DRAM") as dram, tc.tile_pool(
            name="const", bufs=1
        ) as const:
            # -- collective buffers (phi gathered in two b-halves so the
            # first AllGather overlaps the second half of phase 1) --------
            phi_cont_a = dram.tile([B_LOC // 2, MM_PAD], MM_DT)
            phi_cont_b = dram.tile([B_LOC // 2, MM_PAD], MM_DT)
            phi_all_a = dram.tile([B // 2, MM_PAD], MM_DT, addr_space="Shared")
            phi_all_b = dram.tile([B // 2, MM_PAD], MM_DT, addr_space="Shared")
            yt_part = dram.tile([HID, B], F32)
            yt_full = dram.tile([HID, B], F32, addr_space="Shared")

            # -- constants ----------------------------------------------
            identf = const.tile([128, 128], F32)
            masks.make_identity(nc, identf[:])
            if MM_DT != F32:
                ident = const.tile([128, 128], MM_DT)
                masks.make_identity(nc, ident[:])
            else:
                ident = identf
            ones_col = const.tile([128, 128], F32)
            nc.gpsimd.memset(ones_col[:], 1.0)
            tail_pat = const.tile([1, MM_PAD - MM], MM_DT)
            nc.gpsimd.memset(tail_pat[:], 0.0)
            eps_col = const.tile([128, 1], F32)
            nc.gpsimd.memset(eps_col[:], EPS)
            normc_col = const.tile([128, 1], F32)
            nc.gpsimd.memset(normc_col[:], NORM_C)

            # ===========================================================
            # phase 0: preload fc0 weights + small tail tensors (no deps,
            # so these DMAs overlap phase-1 compute)
            # ===========================================================
            w_sb = const.tile([128, NI_CHUNKS, O0_LOC], MM_DT)
            for wd in range(n_wdma):
                k0 = wd * W_DMA
                kn = min(W_DMA, NI_CHUNKS - k0)
                nc.scalar.dma_start(
                    w_sb[:, k0 : k0 + kn, :],
                    w0t_d[128 * k0 : 128 * (k0 + kn)].rearrange(
                        "(kc p) o -> p kc o", p=128
                    ),
                )
            fc0b_sb = const.tile([O0_LOC, 1], F32)
            nc.sync.dma_start(fc0b_sb[:], fc0b_d[:])
            w1_sb = const.tile([O0_LOC, HID], F32)
            nc.sync.dma_start(w1_sb[:], w1t_d[:])
            fc1b_sb = const.tile([HID, 1], F32)
            nc.sync.dma_start(fc1b_sb[:], fc1b_d[:])
            w2_sb = const.tile([HID + 1, CLS], F32)
            nc.sync.dma_start(w2_sb[:], w2t_d[:])
            wc_sb = const.tile([4 * CLS + 1, CLS], F32)
            nc.sync.dma_start(wc_sb[:], wct_d[:])
            xm1_sb = const.tile([B, CLS], F32)
            nc.sync.dma_start(xm1_sb[:], x11_d[:])
            xm2_sb = const.tile([B, CLS], F32)
            nc.sync.dma_start(xm2_sb[:], x21_d[:])
            xm3_sb = const.tile([B, CLS], F32)
            nc.sync.dma_start(xm3_sb[:], x31_d[:])
            # pre-staged tail tiles (written once, reused in phase 5)
            yt_aug = const.tile([HID + 1, B], F32)
            nc.vector.tensor_copy(yt_aug[HID : HID + 1, :], ones_col[0:1, 0:B])
            merged = const.tile([B, 4 * CLS], F32)
            nc.vector.tensor_copy(merged[:, CLS : 2 * CLS], xm1_sb[:])
            nc.vector.tensor_copy(merged[:, 2 * CLS : 3 * CLS], xm2_sb[:])
            nc.vector.tensor_copy(merged[:, 3 * CLS : 4 * CLS], xm3_sb[:])

            # ===========================================================
            # phase 1: bilinear + signed sqrt + normalize, per batch
            # ===========================================================
            with tc.tile_pool(name="xt", bufs=2) as xt_pool, tc.tile_pool(
                name="p1sb", bufs=2
            ) as sb, tc.tile_pool(
                name="p1ps", bufs=2, space="PSUM"
            ) as ps, nc.named_scope("p1_bilinear"):

                def p1_mains(b):
                    # xt[p, lc, m] = x[b, 128*lc + p, m]
                    xt = xt_pool.tile([128, LFULL, M], MM_DT, tag="xt")
                    xtt = xt_pool.tile([LTAIL, M], MM_DT, tag="xtt")
                    nc.sync.dma_start(
                        xt[:],
                        xall_d[b][0 : 128 * LFULL].rearrange(
                            "(lc p) m -> p lc m", p=128
                        ),
                    )
                    nc.sync.dma_start(xtt[:], xall_d[b][128 * LFULL : L])

                    # phi row-blocks: A = rows 0:128, A2 = rows 128:147.
                    # Two separate consecutive accumulation passes: mixing
                    # two PSUM accumulation groups stalls the PE on every
                    # matmul (drain + weight reload between groups).
                    pA = ps.tile([128, M], F32, tag="pA", bufs=3)
                    pB = ps.tile([M - 128, M], F32, tag="pB", bufs=3)
                    for lc in range(LFULL + 1):
                        lhs_a = xt[:, lc, 0:128] if lc < LFULL else xtt[:, 0:128]
                        rhs_a = xt[:, lc, :] if lc < LFULL else xtt[:, :]
                        nc.tensor.matmul(
                            pA[:], lhs_a, rhs_a, start=(lc == 0), stop=(lc == LFULL)
                        )
                    for lc in range(LFULL + 1):
                        lhs_b = xt[:, lc, 128:M] if lc < LFULL else xtt[:, 128:M]
                        rhs_a = xt[:, lc, :] if lc < LFULL else xtt[:, :]
                        nc.tensor.matmul(
                            pB[:], lhs_b, rhs_a, start=(lc == 0), stop=(lc == LFULL)
                        )
                    return pA, pB

                def p1_norm(b, pA, pB):
                    # signed sqrt pieces
                    sgnA = sb.tile([128, M], F32, tag="sgnA")
                    absA = sb.tile([128, M], F32, tag="absA")
                    sgnB = sb.tile([M - 128, M], F32, tag="sgnB")
                    absB = sb.tile([M - 128, M], F32, tag="absB")
                    nc.scalar.activation(
                        sgnA[:], pA[:], mybir.ActivationFunctionType.Sign
                    )
                    nc.scalar.activation(
                        absA[:], pA[:], mybir.ActivationFunctionType.Abs
                    )
                    nc.scalar.activation(
                        sgnB[:], pB[:], mybir.ActivationFunctionType.Sign
                    )
                    nc.scalar.activation(
                        absB[:], pB[:], mybir.ActivationFunctionType.Abs
                    )

                    # row sums of |phi| for the normalizer
                    rsA = sb.tile([128, 1], F32, tag="rsA")
                    rsB = sb.tile([M - 128, 1], F32, tag="rsB")
                    nc.vector.reduce_sum(rsA[:], absA[:], axis=mybir.AxisListType.X)
                    nc.vector.reduce_sum(rsB[:], absB[:], axis=mybir.AxisListType.X)

                    # ss = sign * sqrt(|phi| + EPS)
                    sqA = sb.tile([128, M], F32, tag="sqA")
                    sqB = sb.tile([M - 128, M], F32, tag="sqB")
                    nc.scalar.activation(
                        sqA[:],
                        absA[:],
                        mybir.ActivationFunctionType.Sqrt,
                        bias=eps_col[:],
                    )
                    nc.scalar.activation(
                        sqB[:],
                        absB[:],
                        mybir.ActivationFunctionType.Sqrt,
                        bias=eps_col[0 : M - 128],
                    )
                    ssA = sb.tile([128, M], F32, tag="ssA")
                    ssB = sb.tile([M - 128, M], F32, tag="ssB")
                    nc.vector.tensor_mul(ssA[:], sqA[:], sgnA[:])
                    nc.vector.tensor_mul(ssB[:], sqB[:], sgnB[:])

                    # cross-partition sum + broadcast in one accumulation
                    # group: bc[m] = sum_k ones[k, m] * rs[k]
                    bc = ps.tile([128, 1], F32, tag="bc")
                    nc.tensor.matmul(
                        bc[:], ones_col[:, :], rsA[:], start=True, stop=False
                    )
                    nc.tensor.matmul(
                        bc[:], ones_col[0 : M - 128, :], rsB[:], start=False, stop=True
                    )

                    # scale = 1 / sqrt(total + NORM_C)
                    inv = sb.tile([128, 1], F32, tag="inv")
                    nc.scalar.activation(
                        inv[:],
                        bc[:],
                        mybir.ActivationFunctionType.Sqrt,
                        bias=normc_col[:],
                    )
                    scl = sb.tile([128, 1], F32, tag="scl")
                    nc.vector.reciprocal(scl[:], inv[:])

                    # normalized phi, cast to MM_DT for the gather + fc0
                    nA = sb.tile([128, M], MM_DT, tag="nA")
                    nB = sb.tile([M - 128, M], MM_DT, tag="nB")
                    nc.vector.tensor_scalar_mul(nA[:], ssA[:], scl[:])
                    nc.vector.tensor_scalar_mul(nB[:], ssB[:], scl[0 : M - 128])

                    # write phi row (flattened, m-major) + zero pad tail
                    row = (phi_cont_a if b < B_LOC // 2 else phi_cont_b)[
                        b % (B_LOC // 2)
                    ]
                    nc.scalar.dma_start(
                        row[0 : 128 * M].rearrange("(m n) -> m n", n=M), nA[:]
                    )
                    nc.scalar.dma_start(
                        row[128 * M : MM].rearrange("(m n) -> m n", n=M), nB[:]
                    )
                    nc.scalar.dma_start(row[MM:MM_PAD], tail_pat[0, :])

                # 1-batch software pipeline: batch b's norm chain is issued
                # after batch b+1's matmuls, so the PE stream never stalls
                # waiting for the ACT/DVE chain
                prev = None
                for b in range(B_LOC):
                    cur = (b, *p1_mains(b))
                    if prev is not None:
                        p1_norm(*prev)
                    prev = cur
                p1_norm(*prev)

            # ===========================================================
            # phase 2: AllGather phi
            # ===========================================================
            with nc.named_scope("p2_allgather"):
                nc.gpsimd.collective_compute(
                    "AllGather",
                    mybir.AluOpType.bypass,
                    replica_groups=[CORE_IDS],
                    ins=[phi_cont_a.opt()],
                    outs=[phi_all_a.opt()],
                )
                nc.gpsimd.collective_compute(
                    "AllGather",
                    mybir.AluOpType.bypass,
                    replica_groups=[CORE_IDS],
                    ins=[phi_cont_b.opt()],
                    outs=[phi_all_b.opt()],
                )

            # ===========================================================
            # phase 3: transpose phi, fc0 (o-sharded), fc1 partial
            # ===========================================================
            with tc.tile_pool(name="p3sb", bufs=1) as sb3, tc.tile_pool(
                name="p3ps", bufs=2, space="PSUM"
            ) as ps3, tc.tile_pool(
                name="p3ph", bufs=1, space="PSUM"
            ) as psh, nc.named_scope("p3_fc0"):
                # phiT[p, k, j] = phi^T[128k + p, j] via xbar DMA-transpose,
                # in k-ranges per b-half so fc0 can start on early chunks
                phiT = sb3.tile([128, NI_CHUNKS, B], MM_DT)
                TK = 34
                for h, src_half in enumerate((phi_all_a, phi_all_b)):
                    for k0 in range(0, NI_CHUNKS, TK):
                        kk = min(TK, NI_CHUNKS - k0)
                        nc.sync.dma_start_transpose(
                            phiT[:, k0 : k0 + kk, 32 * h : 32 * (h + 1)],
                            src_half[:, 128 * k0 : 128 * (k0 + kk)],
                        )

                # fc0: h^T (128 o x 64 b), accumulate over 169 i-chunks
                ph = psh.tile([O0_LOC, B], F32)
                for k in range(NI_CHUNKS):
                    nc.tensor.matmul(
                        ph[:],
                        w_sb[:, k, :],
                        phiT[:, k, :],
                        start=(k == 0),
                        stop=(k == NI_CHUNKS - 1),
                    )

                # h = ph + fc0_b (exact fp32 bias add on the PSUM copy-out)
                h_sb = sb3.tile([O0_LOC, B], F32)
                nc.scalar.activation(
                    h_sb[:],
                    ph[:],
                    mybir.ActivationFunctionType.Identity,
                    bias=fc0b_sb[:],
                )

                # fc1 partial: y^T = w1t_shard.T @ h^T_shard
                py = ps3.tile([HID, B], F32, tag="py", bufs=1)
                nc.tensor.matmul(py[:], w1_sb[:], h_sb[:], start=True, stop=True)
                yt_sb = sb3.tile([HID, B], F32)
                nc.vector.tensor_copy(yt_sb[:], py[:])
                nc.sync.dma_start(yt_part[:], yt_sb[:])

            # ===========================================================
            # phase 4: AllReduce y^T partials
            # ===========================================================
            with nc.named_scope("p4_allreduce"):
                nc.gpsimd.collective_compute(
                    "AllReduce",
                    mybir.AluOpType.add,
                    replica_groups=[CORE_IDS],
                    ins=[yt_part.opt()],
                    outs=[yt_full.opt()],
                )

            # ===========================================================
            # phase 5: replicated tail
            # ===========================================================
            with tc.tile_pool(name="p5sb", bufs=1) as sb5, tc.tile_pool(
                name="p5ps", bufs=1, space="PSUM"
            ) as ps5, nc.named_scope("p5_tail"):
                # y^T + fc1_b (ones row pre-staged in phase 0)
                ytr = sb5.tile([HID, B], F32)
                nc.sync.dma_start(ytr[:], yt_full[:])
                nc.scalar.activation(
                    yt_aug[0:HID, :],
                    ytr[:],
                    mybir.ActivationFunctionType.Identity,
                    bias=fc1b_sb[:],
                )

                plog = ps5.tile([B, CLS], F32, tag="plog")
                nc.tensor.matmul(plog[:], yt_aug[:], w2_sb[:], start=True, stop=True)
                logit_sb = sb5.tile([B, CLS], F32)
                nc.scalar.copy(logit_sb[:], plog[:])
                # merged cols 4:16 pre-staged in phase 0 (x1i host-permuted
                # into gathered batch order); logits read from PSUM on DVE in
                # parallel with the ACT copy above
                nc.vector.tensor_copy(merged[:, 0:CLS], plog[:])
                # partition j holds global batch 8*(j%32//4) + 4*(j//32) + j%4
                # (b-halves gathered separately); undo it on the DMA write
                lview = logits_d.rearrange("(s e bl) c -> s e bl c", s=8, e=2)
                nc.sync.dma_start(lview[:, 0], logit_sb[0:32, :])
                nc.sync.dma_start(lview[:, 1], logit_sb[32:B, :])

                # softmax over the 16 features (free dim).  No max-subtract:
                # |merged| <= ~6 here, exp() is safely in range, and softmax
                # is shift-invariant so the result matches the reference.
                esb = sb5.tile([B, 4 * CLS], F32)
                ssum = sb5.tile([B, 1], F32)
                nc.scalar.activation(
                    esb[:],
                    merged[:],
                    mybir.ActivationFunctionType.Exp,
                    accum_out=ssum[:],
                )
                rinv = sb5.tile([B, 1], F32)
                nc.vector.reciprocal(rinv[:], ssum[:])

                # softmax result with a ones column appended (becomes the
                # bias row after the transpose)
                smx = sb5.tile([B, 4 * CLS + 1], F32)
                nc.vector.tensor_scalar_mul(smx[:, 0 : 4 * CLS], esb[:], rinv[:])
                nc.vector.tensor_copy(
                    smx[:, 4 * CLS : 4 * CLS + 1], ones_col[0:B, 0:1]
                )

                # x_merge = smx @ cls_w.T + cls_b  (via transposed smx + aug)
                pmt = ps5.tile([4 * CLS + 1, B], F32, tag="pmt")
                nc.tensor.transpose(pmt[:], smx[:], identf[0:B, 0:B])
                mt_aug = sb5.tile([4 * CLS + 1, B], F32)
                nc.scalar.copy(mt_aug[:], pmt[:])

                pxm = ps5.tile([B, CLS], F32, tag="pxm")
                nc.tensor.matmul(pxm[:], mt_aug[:], wc_sb[:], start=True, stop=True)
                xm_sb = sb5.tile([B, CLS], F32)
                nc.scalar.copy(xm_sb[:], pxm[:])
                xview = xmerge_d.rearrange("(s e bl) c -> s e bl c", s=8, e=2)
                nc.sync.dma_start(xview[:, 0], xm_sb[0:32, :])
                nc.sync.dma_start(xview[:, 1], xm_sb[32:B, :])

    return nc


_NC_CACHE = None


def _get_nc():
    global _NC_CACHE
    if _NC_CACHE is None:
        _NC_CACHE = _build_nc()
    return _NC_CACHE


def _make_in_maps(inputs):
    np_mm = np.dtype(mybir.dt.np(MM_DT))

    x1 = np.ascontiguousarray(inputs["x1"], dtype=np.float32).reshape(B, C, L)
    x2 = np.ascontiguousarray(inputs["x2"], dtype=np.float32).reshape(B, C, L)
    x3 = np.ascontiguousarray(inputs["x3"], dtype=np.float32).reshape(B, C, L)
    # (B, L, M) concatenated + transposed + cast
    xall = np.concatenate([x1, x2, x3], axis=1).transpose(0, 2, 1).astype(np_mm)
    # gathered batch order: columns j map to global batch
    # 8*(j%32//4) + 4*(j//32) + j%4
    perm = np.array(
        [8 * (s) + 4 * e + bl for e in range(2) for s in range(8) for bl in range(4)],
        dtype=np.int64,
    )
    x11 = np.ascontiguousarray(inputs["x11"], dtype=np.float32)[perm]
    x21 = np.ascontiguousarray(inputs["x21"], dtype=np.float32)[perm]
    x31 = np.ascontiguousarray(inputs["x31"], dtype=np.float32)[perm]
    fc0_w = np.asarray(inputs["fc0_w"], dtype=np.float32)
    fc0_b = np.asarray(inputs["fc0_b"], dtype=np.float32)
    fc1_w = np.asarray(inputs["fc1_w"], dtype=np.float32)
    fc1_b = np.asarray(inputs["fc1_b"], dtype=np.float32)
    fc2_w = np.asarray(inputs["fc2_w"], dtype=np.float32)
    fc2_b = np.asarray(inputs["fc2_b"], dtype=np.float32)
    cls_w = np.asarray(inputs["cls_w"], dtype=np.float32)
    cls_b = np.asarray(inputs["cls_b"], dtype=np.float32)

    w0t = np.zeros((MM_PAD, O0), dtype=np_mm)
    w0t[:MM] = fc0_w.T.astype(np_mm)
    w1t = np.ascontiguousarray(fc1_w.T)  # (1024, 64)
    fc1b = np.ascontiguousarray(fc1_b.reshape(HID, 1))
    w2t = np.ascontiguousarray(
        np.concatenate([fc2_w.T, fc2_b.reshape(1, CLS)], axis=0)
    )
    wct = np.ascontiguousarray(
        np.concatenate([cls_w.T, cls_b.reshape(1, CLS)], axis=0)
    )

    in_maps = []
    for c in range(N_CORES):
        sl = slice(B_LOC * c, B_LOC * (c + 1))
        ol = slice(O0_LOC * c, O0_LOC * (c + 1))
        in_maps.append(
            {
                "xall": np.ascontiguousarray(xall[sl]),
                "x11": x11,
                "x21": x21,
                "x31": x31,
                "w0t": np.ascontiguousarray(w0t[:, ol]),
                "fc0b": np.ascontiguousarray(fc0_b[ol].reshape(O0_LOC, 1)),
                "w1t": np.ascontiguousarray(w1t[ol]),
                "fc1b": fc1b,
                "w2t": w2t,
                "wct": wct,
            }
        )
    return in_maps


def run(inputs, trace=False, **kwargs):
    nc = _get_nc()
    in_maps = _make_in_maps(inputs)
    res = run_bass_kernel_spmd(nc, in_maps, CORE_IDS, trace=trace, **kwargs)
    out = res.results[0]
    logits = np.asarray(out["logits"], dtype=np.float32)
    x_merge = np.asarray(out["x_merge"], dtype=np.float32)
    return (logits, x_merge), res


def kernel(**inputs):
    (logits, x_merge), _ = run(inputs, trace=False)
    return logits, x_merge



# revision 19
# speedup vs baseline: 1.3518x; 1.3518x over previous
"""Bilinear pooling kernel for 8 Trainium2 NeuronCores (Bass/Tile).

Math (matches the jax reference):
  x = concat([x1, x2, x3], channel) -> (B=64, M=147, L=3136)
  phi_b = x_b @ x_b.T                              (147, 147), symmetric
  phi = sign(phi) * sqrt(|phi| + EPS)              (signed sqrt)
  phi = phi / sqrt(sum(phi^2 + EPS) + 1.0)         (per-batch normalize)
  h = phi_vec @ fc0_w.T + fc0_b                    (64, 1024)
  y = h @ fc1_w.T + fc1_b                          (64, 64)
  logits = y @ fc2_w.T + fc2_b                     (64, 4)
  merged = softmax(concat([logits, x11, x21, x31]))
  x_merge = merged @ cls_w.T + cls_b               (64, 4)

Distribution (v2):
  - phase 1 is batch-parallel (8 batches/core).  Because phi is symmetric,
    only rows 0:128 (A block, 128x147) and the 19x19 diagonal block are
    computed; the mirror block is recovered by folding fc0's weights
    host-side.  The 19x19 block is built from 5 stacked matmuls (5 l-chunks
    side by side in the PE array) instead of 25 thin ones.
  - normalization is deferred: phase 1 ships UNNORMALIZED signed-sqrt phi
    (fp16) plus per-batch |phi| totals; the 1/sqrt(total+C) scale is applied
    to the 4-wide logits after the final AllReduce (everything in between is
    linear in phi).
  - fc0 is contraction(i)-sharded: an AllToAll gives each core a 2432-wide
    i-slice of every batch's phi vector (~150 KB per collective vs 2.8 MB
    for the old AllGather).  Each core computes partial h (1024) -> partial
    y (64) -> partial z = W2 y (4x64); one tiny AllReduce of (4,64)+totals
    finishes the linear chain, then the softmax tail runs replicated.
  - all DRAM inputs are laid out host-side in exact SBUF layout so every
    DMA moves contiguous multi-KB partition lines.
"""

import sys

sys.path.insert(0, "/opt/trn_rl_repo")

import numpy as np

import concourse.bass as bass
import concourse.tile as tile
from concourse import masks, mybir
from concourse.bass_utils import run_bass_kernel_spmd
import bass_rust
from bass_rust import ScopedClock

# ---------------------------------------------------------------------------
# Workaround: this toolchain's walrus accepts only ONE semaphore wait per
# instruction, but Tile can attach several.  Split excess waits onto
# same-engine nops placed immediately before the instruction (same engine
# => executed in order, so synchronization semantics are unchanged).
# ---------------------------------------------------------------------------
_MAX_WAITS = 1
_ws_counter = [0]


def _split_excess_waits(obb):
    for bb, insts in list(obb.items()):
        new_list = []
        for inst in insts:
            info = inst.sync_info
            if info is not None and len(info.on_wait) > _MAX_WAITS:
                waits = list(info.on_wait)
                excess = waits[:-_MAX_WAITS]
                keep = waits[-_MAX_WAITS:]
                for i in range(0, len(excess), _MAX_WAITS):
                    _ws_counter[0] += 1
                    nop = mybir.InstNoOp(
                        name=f"WS-{_ws_counter[0]}",
                        sync_info=bass_rust.SyncInfo(
                            on_wait=excess[i : i + _MAX_WAITS], on_update=[]
                        ),
                        bass_nofuse=True,
                        engine=inst.engine,
                    )
                    new_list.append(nop)
                inst.sync_info = bass_rust.SyncInfo(
                    on_wait=keep, on_update=list(info.on_update)
                )
            new_list.append(inst)
        obb[bb] = new_list


_RealTCW = tile.TileClockWait


class _TCWWrapper:
    def __init__(self, *args, **kwargs):
        self._inner = _RealTCW(*args, **kwargs)
        self._obb = (
            args[1] if len(args) > 1 else kwargs["ordered_instructions_by_block"]
        )

    def __getattr__(self, name):
        return getattr(self._inner, name)

    def assign_waits(self, bb_name):
        self._inner.assign_waits(bb_name)
        _split_excess_waits(self._obb)


tile.TileClockWait = _TCWWrapper


def _split_drain_and_barrier(self, tick_clock, wait_clock):
    nc = self.nc
    drain_inst = nc.sync.drain()
    wait_clock.add_sem_waits(
        drain_inst.ins, ScopedClock({None: tick_clock.global_clock})
    )
    info = drain_inst.ins.sync_info
    if info is not None and len(info.on_wait) > _MAX_WAITS:
        waits = list(info.on_wait)
        drain_inst.ins.sync_info = bass_rust.SyncInfo(
            on_wait=waits[:_MAX_WAITS], on_update=list(info.on_update)
        )
        rest = waits[_MAX_WAITS:]
        while rest:
            chunk, rest = rest[:_MAX_WAITS], rest[_MAX_WAITS:]
            nop_inst = nc.sync.nop(nofuse=True, hint="tail_drain_split")
            nop_inst.ins.sync_info = bass_rust.SyncInfo(on_wait=chunk, on_update=[])
    nc.all_engine_barrier()
    assert self.sems is not None
    popped = nc._tile_sem_poison_stack.pop()
    assert popped is self._sem_poison
    nc.clear_and_free_semaphores(list(self.sems.allocated().values()))
    nc.all_engine_barrier()


tile.TileContext._drain_and_barrier = _split_drain_and_barrier

# ---------------------------------------------------------------------------
# Problem constants (hardcoded per the spec)
# ---------------------------------------------------------------------------
N_CORES = 8
CORE_IDS = list(range(N_CORES))
B = 64
B_LOC = B // N_CORES  # 8 batches per core
C = 49
L = 3136  # 56*56
LCH = 25  # l-chunks of 128 (last one zero-padded: 3136 = 24*128 + 64)
M = 147  # 3*49 channels
O0 = 1024  # fc0 out features
HID = 64
CLS = 4
EPS = 1e-8
MM = M * M
# normalizer: sum(phi_ss^2 + EPS) + 1.0 == sum|phi| + 2*MM*EPS + 1.0
NORM_C = float(2 * MM * EPS + 1.0)
TOT_SCALE = 256.0  # |phi| totals are carried as fp16 scaled by 1/256

RA = 16  # A-block rows per destination core
RB = 3  # padded-B rows per destination core (B 19x19 padded to 24 rows)
SLICE = 2432  # = 19*128 per-batch per-dest phi slice (2352 A + 57 B + 23 pad)
KCH = SLICE // 128  # 19 fc0 contraction chunks
NB = 8  # fc0 output blocks of 128

F32 = mybir.dt.float32
F16 = mybir.dt.float16


def _build_nc():
    nc = bass.Bass()

    # -- external I/O ------------------------------------------------------
    # all big tensors arrive in exact SBUF layout (partition-major).
    xall_d = nc.dram_tensor("xall", [B_LOC, 128, LCH * M], F16, kind="ExternalInput")
    w0_d = nc.dram_tensor("w0f", [128, KCH * O0], F16, kind="ExternalInput")
    w1_d = nc.dram_tensor("w1t", [128, NB * HID], F32, kind="ExternalInput")
    w2_d = nc.dram_tensor("w2t", [HID, CLS], F32, kind="ExternalInput")
    wct_d = nc.dram_tensor("wct", [4 * CLS + 1, CLS], F32, kind="ExternalInput")
    kb_d = nc.dram_tensor("kb", [B, CLS], F32, kind="ExternalInput")
    xm_d = nc.dram_tensor("xm", [B, 3 * CLS], F32, kind="ExternalInput")
    logits_d = nc.dram_tensor("logits", [B, CLS], F32, kind="ExternalOutput")
    xmerge_d = nc.dram_tensor("x_merge", [B, CLS], F32, kind="ExternalOutput")

    HB = B_LOC // 2  # batches per exchange half
    IN_H = HB * SLICE  # 9728 elems per dest row
    # the 8 per-batch |phi| totals ride inside the 23-elem pad of half 1's
    # last batch slice (W rows there are zero, so fc0 ignores them)
    TOT_OFF = 3 * SLICE + RA * M + RB * 19

    with tile.TileContext(nc) as tc:
        with tc.tile_pool(name="dram", bufs=1, space="DRAM") as dram, tc.tile_pool(
            name="const", bufs=1
        ) as const:
            a2a_in0 = dram.tile([N_CORES, IN_H], F16)
            a2a_in1 = dram.tile([N_CORES, IN_H], F16)
            recv0 = dram.tile([N_CORES, IN_H], F16)
            recv1 = dram.tile([N_CORES, IN_H], F16)
            ar_in = dram.tile([CLS, B], F32)
            ar_out = dram.tile([CLS, B], F32, addr_space="Shared")

            # -- constants ----------------------------------------------
            identf = const.tile([128, 128], F32)
            masks.make_identity(nc, identf[:])
            ones16 = const.tile([128, 8], F16)
            nc.gpsimd.memset(ones16[:], 1.0)
            eps_col = const.tile([128, 1], F32)
            nc.gpsimd.memset(eps_col[:], EPS)
            normc_col = const.tile([B, 1], F32)
            nc.gpsimd.memset(normc_col[:], NORM_C)
            zpad = const.tile([64, SLICE - RA * M - RB * 19], F16)
            nc.gpsimd.memset(zpad[:], 0.0)

            # ===========================================================
            # phase 0: input loads.  x batches first (they gate phase 1),
            # then the fc0 weight slab (only needed ~15us later), all on
            # the sync engine so per-queue FIFO keeps that priority.
            # ===========================================================
            xt = const.tile([128, B_LOC, LCH, M], F16)
            for b in range(B_LOC):
                nc.sync.dma_start(
                    xt[:, b], xall_d[b].rearrange("p (lc m) -> p lc m", lc=LCH)
                )
            w_sb = const.tile([128, KCH, O0], F16)
            for kg in range(4):
                k0, k1 = 5 * kg, min(5 * (kg + 1), KCH)
                nc.sync.dma_start(
                    w_sb[:, k0:k1],
                    w0_d[:, k0 * O0 : k1 * O0].rearrange("p (k o) -> p k o", o=O0),
                )
            w1_sb = const.tile([128, NB, HID], F32)
            nc.scalar.dma_start(
                w1_sb[:], w1_d.rearrange("p (n h) -> p n h", h=HID)
            )
            w2_sb = const.tile([HID, CLS], F32)
            nc.scalar.dma_start(w2_sb[:], w2_d[:])
            wc_sb = const.tile([4 * CLS + 1, CLS], F32)
            nc.scalar.dma_start(wc_sb[:], wct_d[:])
            kb_sb = const.tile([B, CLS], F32)
            nc.scalar.dma_start(kb_sb[:], kb_d[:])
            xm_sb = const.tile([B, 3 * CLS], F32)
            nc.scalar.dma_start(xm_sb[:], xm_d[:])
            merged = const.tile([B, 4 * CLS], F32)
            nc.vector.tensor_copy(merged[:, CLS:], xm_sb[:])
            # zero the 23-elem pad of every (dest, batch) slice once
            nc.scalar.dma_start(
                a2a_in0.rearrange("d (b i) -> (d b) i", b=HB)[:, RA * M + RB * 19 :],
                zpad[0:32],
            )
            nc.scalar.dma_start(
                a2a_in1.rearrange("d (b i) -> (d b) i", b=HB)[
                    :, RA * M + RB * 19 :
                ],
                zpad[32:64],
            )

            # per-batch |phi| row sums, accumulated across phase 1
            rsum = const.tile([128, B_LOC, 2], F32)
            rsumB = const.tile([M - 128, B_LOC], F32)
            # padded signed-sqrt B block; rows 19:24 zeroed once via DMA
            # (engine APs must start at a 32-aligned partition)
            nBp = const.tile([8 * RB, 19], F16)
            nc.sync.dma_start(nBp[19 : 8 * RB, :], zpad[0 : 8 * RB - 19, 0:19])
            # B columns repacked at 32-col pitch so the per-chunk diagonal
            # blocks land on 32-aligned partitions; cols 19:32 stay zero
            bpack = const.tile([128, LCH, 32], F16)
            nc.vector.memset(bpack[:], 0.0)

            # ===========================================================
            # phase 1: bilinear + signed sqrt, per batch (normalization
            # deferred to the tail).  PE stream has NO mid-stream stalls.
            # ===========================================================
            with tc.tile_pool(name="p1sb", bufs=2) as sb, tc.tile_pool(
                name="p1psA", bufs=3, space="PSUM"
            ) as psA, tc.tile_pool(
                name="p1psB", bufs=2, space="PSUM"
            ) as psB, nc.named_scope("p1_bilinear"):
                for b in range(B_LOC):
                    half, bl = divmod(b, HB)
                    in_h = a2a_in0 if half == 0 else a2a_in1

                    # A block: phi rows 0:128 x cols 0:147
                    pA = psA.tile([128, M], F32, tag="pA")
                    for lc in range(LCH):
                        nc.tensor.matmul(
                            pA[:],
                            xt[:, b, lc, 0:128],
                            xt[:, b, lc, :],
                            start=(lc == 0),
                            stop=(lc == LCH - 1),
                        )
                    # B diagonal 19x19: 4 l-chunks stacked side by side at
                    # 32-col pitch; diag 32-blocks of the 128x128 result
                    # hold the per-chunk contributions, summed below on DVE.
                    nc.vector.tensor_copy(bpack[:, :, 0:19], xt[:, b, :, 128:M])
                    pB5 = psB.tile([128, 128], F32, tag="pB5")
                    for s in range(6):
                        ap = bpack[:, 4 * s : 4 * s + 4, :]
                        nc.tensor.matmul(
                            pB5[:], ap, ap, start=(s == 0), stop=False
                        )
                    ap = bpack[:, 24, :]
                    nc.tensor.matmul(
                        pB5[0:32, 0:32], ap, ap, start=False, stop=True
                    )

                    # norm chain (ACT + DVE only; PE keeps streaming)
                    sgnA = sb.tile([128, M], F32, tag="sgnA")
                    absA = sb.tile([128, M], F32, tag="absA")
                    nc.scalar.activation(
                        sgnA[:], pA[:], mybir.ActivationFunctionType.Sign
                    )
                    nc.scalar.activation(
                        absA[:], pA[:], mybir.ActivationFunctionType.Abs
                    )
                    nc.vector.reduce_sum(
                        rsum[:, b, 0:1], absA[:], axis=mybir.AxisListType.X
                    )
                    # mirror columns 128:147 count twice in the full |phi| sum
                    nc.vector.reduce_sum(
                        rsum[:, b, 1:2], absA[:, 128:M], axis=mybir.AxisListType.X
                    )
                    sqA = sb.tile([128, M], F32, tag="sqA")
                    nc.scalar.activation(
                        sqA[:],
                        absA[:],
                        mybir.ActivationFunctionType.Sqrt,
                        bias=eps_col[:],
                    )
                    nA = sb.tile([128, M], F16, tag="nA")
                    nc.vector.tensor_mul(nA[:], sqA[:], sgnA[:])
                    nc.scalar.dma_start(
                        in_h[:, bl * SLICE : bl * SLICE + RA * M].rearrange(
                            "d (r m) -> d r m", r=RA
                        ),
                        nA[:],
                    )

                    bsum = sb.tile([19, 19], F32, tag="bsum")
                    nc.vector.tensor_copy(bsum[:], pB5[0:19, 0:19])
                    nc.vector.tensor_add(bsum[:], bsum[:], pB5[32:51, 32:51])
                    nc.vector.tensor_add(bsum[:], bsum[:], pB5[64:83, 64:83])
                    nc.vector.tensor_add(bsum[:], bsum[:], pB5[96:115, 96:115])
                    sgnB = sb.tile([19, 19], F32, tag="sgnB")
                    absB = sb.tile([19, 19], F32, tag="absB")
                    nc.scalar.activation(
                        sgnB[:], bsum[:], mybir.ActivationFunctionType.Sign
                    )
                    nc.scalar.activation(
                        absB[:], bsum[:], mybir.ActivationFunctionType.Abs
                    )
                    nc.vector.reduce_sum(
                        rsumB[:, b : b + 1], absB[:], axis=mybir.AxisListType.X
                    )
                    sqB = sb.tile([19, 19], F32, tag="sqB")
                    nc.scalar.activation(
                        sqB[:],
                        absB[:],
                        mybir.ActivationFunctionType.Sqrt,
                        bias=eps_col[0:19],
                    )
                    nc.vector.tensor_mul(nBp[0:19, :], sqB[:], sgnB[:])
                    nc.scalar.dma_start(
                        in_h[
                            :, bl * SLICE + RA * M : bl * SLICE + RA * M + RB * 19
                        ].rearrange("d (s c) -> d s c", s=RB),
                        nBp[:],
                    )

                    if b == HB - 1:
                        with nc.named_scope("p2_a2a0"):
                            nc.gpsimd.collective_compute(
                                "AllToAll",
                                mybir.AluOpType.bypass,
                                replica_groups=[CORE_IDS],
                                ins=[a2a_in0.opt()],
                                outs=[recv0.opt()],
                            )

                # per-batch |phi| totals: cross-partition sum via one
                # fp16 ones-matmul (PE is idle by now), scaled by 1/256
                # to fit fp16 on the wire.
                rs_sum = sb.tile([128, B_LOC], F32, tag="rs_sum")
                nc.vector.tensor_add(rs_sum[:], rsum[:, :, 0], rsum[:, :, 1])
                rs16 = sb.tile([128, B_LOC], F16, tag="rs16")
                nc.vector.tensor_scalar_mul(rs16[:], rs_sum[:], 1.0 / TOT_SCALE)
                rsB16 = sb.tile([M - 128, B_LOC], F16, tag="rsB16")
                nc.vector.tensor_scalar_mul(rsB16[:], rsumB[:], 1.0 / TOT_SCALE)
                tot_ps = psB.tile([8, 8], F32, tag="tot")
                nc.tensor.matmul(
                    tot_ps[:], ones16[:, :], rs16[:], start=True, stop=False
                )
                nc.tensor.matmul(
                    tot_ps[:],
                    ones16[0 : M - 128, :],
                    rsB16[:],
                    start=False,
                    stop=True,
                )
                tot16 = sb.tile([8, 8], F16, tag="tot16")
                nc.scalar.copy(tot16[:], tot_ps[:])
                nc.scalar.dma_start(
                    a2a_in1[:, TOT_OFF : TOT_OFF + B_LOC], tot16[:]
                )

            with nc.named_scope("p2_a2a1"):
                nc.gpsimd.collective_compute(
                    "AllToAll",
                    mybir.AluOpType.bypass,
                    replica_groups=[CORE_IDS],
                    ins=[a2a_in1.opt()],
                    outs=[recv1.opt()],
                )

            # ===========================================================
            # phase 3: transpose received phi slices, fc0/fc1/fc2 partials
            # column j of phiT holds batch 8*(j%32//4) + 4*(j//32) + j%4
            # ===========================================================
            with tc.tile_pool(name="p3sb", bufs=1) as sb3, tc.tile_pool(
                name="p3ps", bufs=1, space="PSUM"
            ) as ps3, nc.named_scope("p3_fc0"):
                phiT = sb3.tile([128, KCH, B], F16)
                nc.sync.dma_start_transpose(
                    phiT[:, :, 0:32],
                    recv0.rearrange("d (b i) -> (d b) i", b=HB),
                )
                nc.sync.dma_start_transpose(
                    phiT[:, :, 32:B],
                    recv1.rearrange("d (b i) -> (d b) i", b=HB),
                )

                h_sb = sb3.tile([128, NB, B], F32)
                for ob in range(NB):
                    ph = ps3.tile([128, B], F32, tag=f"h{ob % 2}", bufs=2)
                    for k in range(KCH):
                        nc.tensor.matmul(
                            ph[:],
                            w_sb[:, k, 128 * ob : 128 * (ob + 1)],
                            phiT[:, k, :],
                            start=(k == 0),
                            stop=(k == KCH - 1),
                        )
                    nc.scalar.copy(h_sb[:, ob, :], ph[:])

                py = ps3.tile([HID, B], F32, tag="py")
                for ob in range(NB):
                    nc.tensor.matmul(
                        py[:],
                        w1_sb[:, ob, :],
                        h_sb[:, ob, :],
                        start=(ob == 0),
                        stop=(ob == NB - 1),
                    )
                y_sb = sb3.tile([HID, B], F32)
                nc.vector.tensor_copy(y_sb[:], py[:])
                pz = ps3.tile([CLS, B], F32, tag="pz")
                nc.tensor.matmul(pz[:], w2_sb[:], y_sb[:], start=True, stop=True)
                z_sb = sb3.tile([CLS, B], F32)
                nc.scalar.copy(z_sb[:], pz[:])
                nc.scalar.dma_start(ar_in[:], z_sb[:])

            with nc.named_scope("p4_allreduce"):
                nc.gpsimd.collective_compute(
                    "AllReduce",
                    mybir.AluOpType.add,
                    replica_groups=[CORE_IDS],
                    ins=[ar_in.opt()],
                    outs=[ar_out.opt()],
                )

            # ===========================================================
            # phase 5: replicated tail (scale, bias, softmax, cls head)
            # ===========================================================
            with tc.tile_pool(name="p5sb", bufs=1) as sb5, tc.tile_pool(
                name="p5ps", bufs=1, space="PSUM"
            ) as ps5, nc.named_scope("p5_tail"):
                z4 = sb5.tile([CLS, B], F32)
                nc.sync.dma_start(z4[:], ar_out[:])
                tot64 = sb5.tile([1, B], F16)
                # tot64 col j = total of batch(j): recv1[s][TOT_OFF + 4e + bl]
                for e in range(2):
                    nc.sync.dma_start(
                        tot64[0:1, 32 * e : 32 * e + 32],
                        recv1[:, TOT_OFF + 4 * e : TOT_OFF + 4 * e + 4],
                    )
                ts32 = sb5.tile([1, B], F32)
                nc.vector.tensor_copy(ts32[:], tot64[:])

                ptz = ps5.tile([B, CLS], F32, tag="ptz")
                nc.tensor.transpose(ptz[:], z4[:], identf[0:CLS, 0:CLS])
                ptt = ps5.tile([B, 1], F32, tag="ptt")
                nc.tensor.transpose(ptt[:], ts32[:], identf[0:1, 0:1])

                sq = sb5.tile([B, 1], F32)
                nc.scalar.activation(
                    sq[:],
                    ptt[:],
                    mybir.ActivationFunctionType.Sqrt,
                    bias=normc_col[:],
                    scale=TOT_SCALE,
                )
                sinv = sb5.tile([B, 1], F32)
                nc.vector.reciprocal(sinv[:], sq[:])
                logit_sb = sb5.tile([B, CLS], F32)
                nc.vector.tensor_scalar_mul(logit_sb[:], ptz[:], sinv[:])
                nc.vector.tensor_add(logit_sb[:], logit_sb[:], kb_sb[:])
                # partition j holds batch 8*(j%32//4) + 4*(j//32) + j%4
                lview = logits_d.rearrange("(s e bl) c -> s e bl c", s=8, e=2)
                nc.sync.dma_start(lview[:, 0], logit_sb[0:32, :])
                nc.sync.dma_start(lview[:, 1], logit_sb[32:B, :])

                nc.vector.tensor_copy(merged[:, 0:CLS], logit_sb[:])
                # softmax over 16 features; no max-subtract (|merged| <= ~6)
                esb = sb5.tile([B, 4 * CLS], F32)
                ssum = sb5.tile([B, 1], F32)
                nc.scalar.activation(
                    esb[:],
                    merged[:],
                    mybir.ActivationFunctionType.Exp,
                    accum_out=ssum[:],
                )
                rinv = sb5.tile([B, 1], F32)
                nc.vector.reciprocal(rinv[:], ssum[:])
                smx = sb5.tile([B, 4 * CLS + 1], F32)
                nc.vector.tensor_scalar_mul(smx[:, 0 : 4 * CLS], esb[:], rinv[:])
                nc.vector.memset(smx[:, 4 * CLS :], 1.0)

                pmt = ps5.tile([4 * CLS + 1, B], F32, tag="pmt")
                nc.tensor.transpose(pmt[:], smx[:], identf[0:B, 0:B])
                mt = sb5.tile([4 * CLS + 1, B], F32)
                nc.scalar.copy(mt[:], pmt[:])
                pxm = ps5.tile([B, CLS], F32, tag="pxm")
                nc.tensor.matmul(pxm[:], mt[:], wc_sb[:], start=True, stop=True)
                xm_out = sb5.tile([B, CLS], F32)
                nc.scalar.copy(xm_out[:], pxm[:])
                xview = xmerge_d.rearrange("(s e bl) c -> s e bl c", s=8, e=2)
                nc.sync.dma_start(xview[:, 0], xm_out[0:32, :])
                nc.sync.dma_start(xview[:, 1], xm_out[32:B, :])

    return nc


_NC_CACHE = None


def _get_nc():
    global _NC_CACHE
    if _NC_CACHE is None:
        _NC_CACHE = _build_nc()
    return _NC_CACHE


_PREP_CACHE = {}


def _prep_weights(inputs):
    """Host-side weight folding/layout (cached across calls)."""
    key = id(inputs.get("fc0_w"))
    if key in _PREP_CACHE:
        return _PREP_CACHE[key]

    fc0_w = np.asarray(inputs["fc0_w"], dtype=np.float32)
    fc0_b = np.asarray(inputs["fc0_b"], dtype=np.float32)
    fc1_w = np.asarray(inputs["fc1_w"], dtype=np.float32)
    fc1_b = np.asarray(inputs["fc1_b"], dtype=np.float32)
    fc2_w = np.asarray(inputs["fc2_w"], dtype=np.float32)
    fc2_b = np.asarray(inputs["fc2_b"], dtype=np.float32)
    cls_w = np.asarray(inputs["cls_w"], dtype=np.float32)
    cls_b = np.asarray(inputs["cls_b"], dtype=np.float32)

    # symmetry-folded fc0 coefficients
    resh = fc0_w.T.reshape(M, M, O0)  # [m, n, o]
    WA = resh[0:128, :, :].copy()
    WA[:, 128:M, :] += np.transpose(resh[128:M, 0:128, :], (1, 0, 2))
    WBp = np.zeros((8 * RB, 19, O0), dtype=np.float32)
    WBp[0:19] = resh[128:M, 128:M, :]

    w0_cores = []
    for d in range(N_CORES):
        w_slice = np.zeros((SLICE, O0), dtype=np.float32)
        w_slice[0 : RA * M] = WA[RA * d : RA * (d + 1)].reshape(RA * M, O0)
        w_slice[RA * M : RA * M + RB * 19] = WBp[RB * d : RB * (d + 1)].reshape(
            RB * 19, O0
        )
        # device layout [p][k][o], i_local = 128*k + p
        w0_cores.append(
            np.ascontiguousarray(
                w_slice.reshape(KCH, 128, O0)
                .transpose(1, 0, 2)
                .reshape(128, KCH * O0)
                .astype(np.float16)
            )
        )

    w1t = np.ascontiguousarray(
        fc1_w.T.reshape(NB, 128, HID).transpose(1, 0, 2).reshape(128, NB * HID)
    )
    w2t = np.ascontiguousarray(fc2_w.T)
    wct = np.ascontiguousarray(
        np.concatenate([cls_w.T, cls_b.reshape(1, CLS)], axis=0)
    )
    kb = fc2_w @ (fc1_w @ fc0_b + fc1_b) + fc2_b
    kb64 = np.ascontiguousarray(np.broadcast_to(kb, (B, CLS)).copy())

    out = (w0_cores, w1t, w2t, wct, kb64)
    _PREP_CACHE[key] = out
    return out


def _make_in_maps(inputs):
    x1 = np.ascontiguousarray(inputs["x1"], dtype=np.float32).reshape(B, C, L)
    x2 = np.ascontiguousarray(inputs["x2"], dtype=np.float32).reshape(B, C, L)
    x3 = np.ascontiguousarray(inputs["x3"], dtype=np.float32).reshape(B, C, L)
    xc = np.concatenate([x1, x2, x3], axis=1)  # (B, M, L)
    xp = np.zeros((B, M, LCH * 128), dtype=np.float32)
    xp[:, :, 0:L] = xc
    # device layout [b][p][lc*M + m] = x[b, m, 128*lc + p]
    xall = np.ascontiguousarray(
        xp.reshape(B, M, LCH, 128)
        .transpose(0, 3, 2, 1)
        .reshape(B, 128, LCH * M)
        .astype(np.float16)
    )

    w0_cores, w1t, w2t, wct, kb64 = _prep_weights(inputs)

    # phiT column j holds batch 8*(j%32//4) + 4*(j//32) + j%4
    perm = np.array(
        [8 * s + 4 * e + bl for e in range(2) for s in range(8) for bl in range(4)],
        dtype=np.int64,
    )
    xm = np.concatenate(
        [
            np.asarray(inputs["x11"], dtype=np.float32),
            np.asarray(inputs["x21"], dtype=np.float32),
            np.asarray(inputs["x31"], dtype=np.float32),
        ],
        axis=1,
    )[perm]
    xm = np.ascontiguousarray(xm)

    in_maps = []
    for cidx in range(N_CORES):
        sl = slice(B_LOC * cidx, B_LOC * (cidx + 1))
        in_maps.append(
            {
                "xall": np.ascontiguousarray(xall[sl]),
                "w0f": w0_cores[cidx],
                "w1t": w1t,
                "w2t": w2t,
                "wct": wct,
                "kb": kb64,
                "xm": xm,
            }
        )
    return in_maps


def run(inputs, trace=False, **kwargs):
    nc = _get_nc()
    in_maps = _make_in_maps(inputs)
    res = run_bass_kernel_spmd(nc, in_maps, CORE_IDS, trace=trace, **kwargs)
    out = res.results[0]
    logits = np.asarray(out["logits"], dtype=np.float32)
    x_merge = np.asarray(out["x_merge"], dtype=np.float32)
    return (logits, x_merge), res


def kernel(**inputs):
    (logits, x_merge), _ = run(inputs, trace=False)
    return logits, x_merge
